# revision 10
# baseline (speedup 1.0000x reference)
"""Trainium2 Bass kernel for EnhancedGatedFusion (MoE routing, top-2 of 8 experts).

Strategy: data-parallel over tokens across 8 NeuronCores. Each core gets
T=1024 tokens (full weights replicated) and computes:
  router logits (true fp32 matmul - top-2 selection is precision critical),
  top-2 softmax gate weights via dense max/mask trick,
  dense 8-expert MLP (float32r matmuls at full PE rate) accumulated in a
  transposed C^T [D, T] layout so expert bias is per-partition and no
  transpose is needed before the projection matmul,
  projection + residual + RMSNorm in token-major layout.
"""

import sys

for _p in ("/opt/trn_rl_repo",):
    if _p not in sys.path:
        sys.path.insert(0, _p)

from contextlib import ExitStack

import numpy as np

import concourse.bass as bass
import concourse.mybir as mybir
import concourse.tile as tile
from concourse import bacc
from concourse.masks import make_identity

FP32 = mybir.dt.float32
FP32R = mybir.dt.float32r
AX = mybir.AxisListType
ALU = mybir.AluOpType
ACTF = mybir.ActivationFunctionType

EPS = 1e-6
NEG_BIG = -1e30


def _bcast_ap(ap, nparts=128):
    """Partition-broadcast view of a DRAM AP (step-0 partition dim)."""
    return bass.AP(tensor=ap.tensor, offset=ap.offset, ap=[[0, nparts], *ap.ap])


def build_moe_nc(D, E, T, PW=256, trn_type="TRN2"):
    """Emit the per-core MoE program. Returns a compiled Bacc instance.

    D: model dim (multiple of 128); E: num experts; T: tokens per core;
    PW: weight panel width (multiple of 128, >=256 for f32r full rate on proj).
    """
    P = 128
    KO = D // P          # contraction k-tiles
    NTT = T // P         # token tiles of 128
    TOKMM = min(512, T)  # moving-operand token chunk for expert matmuls
    NTH = T // TOKMM     # token chunks
    NCP = D // PW        # weight panels (expert cols / proj cols)
    NCT = PW // P        # col-tiles of 128 per panel

    nc = bacc.Bacc(trn_type, target_bir_lowering=False, debug=False)

    xt = nc.dram_tensor("xt", [D, T], FP32, kind="ExternalInput").ap()
    xr = nc.dram_tensor("xr", [T, D], FP32, kind="ExternalInput").ap()
    router_w = nc.dram_tensor("router_w", [D, E], FP32, kind="ExternalInput").ap()
    router_b = nc.dram_tensor("router_b", [E], FP32, kind="ExternalInput").ap()
    expert_w = nc.dram_tensor("expert_w", [E, D, D], FP32, kind="ExternalInput").ap()
    expert_b = nc.dram_tensor("expert_b", [E, D], FP32, kind="ExternalInput").ap()
    proj_w = nc.dram_tensor("proj_w", [D, D], FP32, kind="ExternalInput").ap()
    proj_b = nc.dram_tensor("proj_b", [D], FP32, kind="ExternalInput").ap()
    norm_w = nc.dram_tensor("norm_w", [D], FP32, kind="ExternalInput").ap()
    out = nc.dram_tensor("out", [T, D], FP32, kind="ExternalOutput").ap()
    fw_dram = nc.dram_tensor("fw_scratch", [E, T], FP32).ap()

    xt_r = xt.rearrange("(ko p) t -> p ko t", p=P)
    rw_r = router_w.rearrange("(ko p) e -> p ko e", p=P)

    with tile.TileContext(nc) as tc, ExitStack() as ctx:
        v = nc.vector
        s = nc.scalar

        big = ctx.enter_context(tc.tile_pool(name="big", bufs=1))
        ct_pool = ctx.enter_context(tc.tile_pool(name="ct_pool", bufs=1))
        w_pool = ctx.enter_context(tc.tile_pool(name="w_pool", bufs=2))
        sil_pool = ctx.enter_context(tc.tile_pool(name="sil_pool", bufs=3))
        small = ctx.enter_context(tc.tile_pool(name="small", bufs=2))
        singles = ctx.enter_context(tc.tile_pool(name="singles", bufs=1))
        xres_pool = ctx.enter_context(tc.tile_pool(name="xres_pool", bufs=2))

        # ---- resident loads ----
        xt_sb = big.tile([P, KO, T], FP32R, tag="big")
        nc.sync.dma_start(out=xt_sb, in_=xt_r.bitcast(FP32R))

        rw_sb = singles.tile([P, KO, E], FP32)
        nc.sync.dma_start(out=rw_sb, in_=rw_r)
        rb_rep = singles.tile([P, E], FP32)
        nc.sync.dma_start(out=rb_rep, in_=_bcast_ap(router_b))
        nw_rep = singles.tile([P, D], FP32)
        nc.sync.dma_start(out=nw_rep, in_=_bcast_ap(norm_w))
        prb_rep = singles.tile([P, D], FP32)
        nc.sync.dma_start(out=prb_rep, in_=_bcast_ap(proj_b))

        identity = singles.tile([P, P], FP32)
        make_identity(nc, identity)
        eps_t = singles.tile([P, 1], FP32)
        v.memset(eps_t, EPS)

        fwT = singles.tile([E, T], FP32)  # gate weights, expert-major
        ct = ct_pool.tile([P, KO, T], FP32R)  # C^T accumulator [D, T]

        # ---- router + top-2 softmax gates ----
        with (
            tc.tile_pool(name="psr", bufs=2, space="PSUM") as psr,
            tc.tile_pool(name="pst", bufs=2, space="PSUM") as pst,
            tc.tile_pool(name="rsm", bufs=2) as rsm,
        ):
            for tt in range(NTT):
                ps_l = psr.tile([P, E], FP32)
                for ko in range(KO):
                    nc.tensor.matmul(
                        ps_l,
                        lhsT=xt_sb[:, ko, tt * P:(tt + 1) * P].bitcast(FP32),
                        rhs=rw_sb[:, ko, :],
                        start=(ko == 0),
                        stop=(ko == KO - 1),
                    )
                logits = rsm.tile([P, E], FP32)
                v.tensor_tensor(out=logits, in0=ps_l, in1=rb_rep, op=ALU.add)
                m1 = rsm.tile([P, 1], FP32)
                v.tensor_reduce(m1, logits, axis=AX.X, op=ALU.max)
                mask1 = rsm.tile([P, E], FP32)
                v.tensor_scalar(mask1, logits, m1, None, op0=ALU.is_ge)
                lg2 = rsm.tile([P, E], FP32)
                v.scalar_tensor_tensor(
                    out=lg2, in0=mask1, scalar=NEG_BIG, in1=logits,
                    op0=ALU.mult, op1=ALU.add,
                )
                m2 = rsm.tile([P, 1], FP32)
                v.tensor_reduce(m2, lg2, axis=AX.X, op=ALU.max)
                mask2 = rsm.tile([P, E], FP32)
                v.tensor_scalar(mask2, lg2, m2, None, op0=ALU.is_ge)
                d21 = rsm.tile([P, 1], FP32)
                v.tensor_tensor(out=d21, in0=m2, in1=m1, op=ALU.subtract)
                e2 = rsm.tile([P, 1], FP32)
                s.activation(e2, d21, ACTF.Exp)
                den = rsm.tile([P, 1], FP32)
                v.tensor_scalar(den, e2, 1.0, None, op0=ALU.add)
                winv = rsm.tile([P, 1], FP32)
                v.reciprocal(winv, den)
                w2 = rsm.tile([P, 1], FP32)
                v.tensor_tensor(out=w2, in0=e2, in1=winv, op=ALU.mult)
                t2 = rsm.tile([P, E], FP32)
                v.tensor_scalar(t2, mask2, w2, None, op0=ALU.mult)
                fw = rsm.tile([P, E], FP32)
                v.scalar_tensor_tensor(
                    out=fw, in0=mask1, scalar=winv, in1=t2,
                    op0=ALU.mult, op1=ALU.add,
                )
                ps_t = pst.tile([E, P], FP32)
                nc.tensor.transpose(ps_t, fw, identity)
                v.tensor_copy(out=fwT[:, tt * P:(tt + 1) * P], in_=ps_t)
            nc.sync.dma_start(out=fw_dram, in_=fwT)

        # ---- expert phase: ct[d, t] = sum_e gate[e,t] * silu(x @ We + be)^T ----
        with tc.tile_pool(name="pse", bufs=6, space="PSUM") as pse:
            for e in range(E):
                fw_rep = sil_pool.tile([P, T], FP32, tag="fwrep", bufs=2)
                nc.sync.dma_start(out=fw_rep, in_=_bcast_ap(fw_dram[e]))
                eb_sb = small.tile([P, KO], FP32)
                nc.sync.dma_start(
                    out=eb_sb, in_=expert_b[e].rearrange("(ko p) -> p ko", p=P)
                )
                we_r = expert_w[e].rearrange("(ko p) c -> p ko c", p=P)
                for cq in range(NCP):
                    wp = w_pool.tile([P, KO, PW], FP32R, tag="wp")
                    nc.sync.dma_start(
                        out=wp, in_=we_r[:, :, cq * PW:(cq + 1) * PW].bitcast(FP32R)
                    )
                    for c2 in range(NCT):
                        colt = cq * NCT + c2
                        for th in range(NTH):
                            ps = pse.tile([P, TOKMM], FP32)
                            for ko in range(KO):
                                nc.tensor.matmul(
                                    ps,
                                    lhsT=wp[:, ko, c2 * P:(c2 + 1) * P],
                                    rhs=xt_sb[:, ko, th * TOKMM:(th + 1) * TOKMM],
                                    start=(ko == 0),
                                    stop=(ko == KO - 1),
                                )
                            sg = sil_pool.tile([P, TOKMM], FP32, tag="sg")
                            s.activation(
                                sg, ps, ACTF.Sigmoid, bias=eb_sb[:, colt:colt + 1]
                            )
                            sil = sil_pool.tile([P, TOKMM], FP32, tag="sil")
                            v.scalar_tensor_tensor(
                                out=sil, in0=ps, scalar=eb_sb[:, colt:colt + 1],
                                in1=sg, op0=ALU.add, op1=ALU.mult,
                            )
                            ct_sl = ct[:, colt, th * TOKMM:(th + 1) * TOKMM]
                            fw_sl = fw_rep[:, th * TOKMM:(th + 1) * TOKMM]
                            if e == 0:
                                v.tensor_tensor(
                                    out=ct_sl, in0=sil, in1=fw_sl, op=ALU.mult
                                )
                            else:
                                v.tensor_tensor(
                                    out=sil, in0=sil, in1=fw_sl, op=ALU.mult
                                )
                                v.tensor_tensor(
                                    out=ct_sl, in0=ct_sl, in1=sil, op=ALU.add
                                )

        # ---- projection + residual into Y (token-major), reusing xt's slot ----
        y_all = big.tile([P, NTT, D], FP32, tag="big")
        pw_r = proj_w.rearrange("(ko p) c -> p ko c", p=P)
        with tc.tile_pool(name="psp", bufs=4, space="PSUM") as psp:
            for pp in range(NCP):
                pwp = w_pool.tile([P, KO, PW], FP32R, tag="wp")
                nc.sync.dma_start(out=pwp, in_=pw_r[:, :, pp * PW:(pp + 1) * PW].bitcast(FP32R))
                for tt in range(NTT):
                    ps_o = psp.tile([P, PW], FP32)
                    for ko in range(KO):
                        nc.tensor.matmul(
                            ps_o,
                            lhsT=ct[:, ko, tt * P:(tt + 1) * P],
                            rhs=pwp[:, ko, :],
                            start=(ko == 0),
                            stop=(ko == KO - 1),
                        )
                    xres = xres_pool.tile([P, PW], FP32)
                    nc.sync.dma_start(
                        out=xres,
                        in_=xr[tt * P:(tt + 1) * P, pp * PW:(pp + 1) * PW],
                    )
                    y_sl = y_all[:, tt, pp * PW:(pp + 1) * PW]
                    v.tensor_tensor(
                        out=y_sl, in0=ps_o, in1=prb_rep[:, pp * PW:(pp + 1) * PW],
                        op=ALU.add,
                    )
                    v.tensor_tensor(out=y_sl, in0=y_sl, in1=xres, op=ALU.add)

        # ---- RMS norm (in place on Y) + store ----
        with tc.tile_pool(name="nsm", bufs=2) as nsm:
            HD = D // 2
            for tt in range(NTT):
                y_t = y_all[:, tt, :]
                sq = nsm.tile([P, HD], FP32, tag="sq", bufs=1)
                ssa = nsm.tile([P, 1], FP32, tag="ssa")
                ssb = nsm.tile([P, 1], FP32, tag="ssb")
                v.scalar_tensor_tensor(
                    out=sq, in0=y_t[:, :HD], scalar=1.0, in1=y_t[:, :HD],
                    op0=ALU.bypass, op1=ALU.mult, accum_out=ssa,
                )
                v.scalar_tensor_tensor(
                    out=sq, in0=y_t[:, HD:], scalar=1.0, in1=y_t[:, HD:],
                    op0=ALU.bypass, op1=ALU.mult, accum_out=ssb,
                )
                ssum = nsm.tile([P, 1], FP32, tag="ssum")
                v.tensor_tensor(out=ssum, in0=ssa, in1=ssb, op=ALU.add)
                rms = nsm.tile([P, 1], FP32, tag="rms")
                s.activation(rms, ssum, ACTF.Sqrt, bias=eps_t, scale=1.0 / D)
                rinv = nsm.tile([P, 1], FP32, tag="rinv")
                v.reciprocal(rinv, rms)
                v.scalar_tensor_tensor(
                    out=y_t, in0=y_t, scalar=rinv, in1=nw_rep,
                    op0=ALU.mult, op1=ALU.mult,
                )
                nc.sync.dma_start(out=out[tt * P:(tt + 1) * P, :], in_=y_t)

    nc.compile()
    return nc


# ---- full-problem entry point ----
_B, _S, _D, _E = 4, 2048, 2048, 8
_NCORES = 8
_T = _B * _S // _NCORES

_nc_cache = None


def _get_nc():
    global _nc_cache
    if _nc_cache is None:
        _nc_cache = build_moe_nc(_D, _E, _T)
    return _nc_cache


def kernel(x, router_w, router_b, expert_w, expert_b, proj_w, proj_b, norm_w):
    from concourse import bass_utils

    x = np.asarray(x, np.float32)
    router_w = np.asarray(router_w, np.float32)
    router_b = np.asarray(router_b, np.float32)
    expert_w = np.asarray(expert_w, np.float32)
    expert_b = np.asarray(expert_b, np.float32)
    proj_w = np.asarray(proj_w, np.float32)
    proj_b = np.asarray(proj_b, np.float32)
    norm_w = np.asarray(norm_w, np.float32)

    nc = _get_nc()
    xf = x.reshape(-1, _D)
    in_maps = []
    for c in range(_NCORES):
        xs = xf[c * _T:(c + 1) * _T]
        in_maps.append({
            "xt": np.ascontiguousarray(xs.T),
            "xr": np.ascontiguousarray(xs),
            "router_w": router_w,
            "router_b": router_b,
            "expert_w": expert_w,
            "expert_b": expert_b,
            "proj_w": proj_w,
            "proj_b": proj_b,
            "norm_w": norm_w,
        })
    res = bass_utils.run_bass_kernel_spmd(nc, in_maps, core_ids=list(range(_NCORES)))
    outs = [res.results[c]["out"] for c in range(_NCORES)]
    return np.concatenate(outs, axis=0).reshape(_B, _S, _D).astype(np.float32)


# revision 14
# speedup vs baseline: 1.0305x; 1.0305x over previous
"""Trainium2 Bass kernel for EnhancedGatedFusion (MoE routing, top-2 of 8 experts).

Strategy: data-parallel over tokens across 8 NeuronCores. Each core gets
T=1024 tokens (full weights replicated) and computes:
  router logits (true fp32 matmul - top-2 selection is precision critical),
  top-2 softmax gate weights via dense max/mask trick,
  dense 8-expert MLP (float32r matmuls at full PE rate) accumulated in a
  transposed C^T [D, T] layout so expert bias is per-partition and no
  transpose is needed before the projection matmul,
  projection + residual + RMSNorm in token-major layout.
"""

import sys

for _p in ("/opt/trn_rl_repo",):
    if _p not in sys.path:
        sys.path.insert(0, _p)

from contextlib import ExitStack

import numpy as np

import concourse.bass as bass
import concourse.mybir as mybir
import concourse.tile as tile
from concourse import bacc
from concourse.masks import make_identity

FP32 = mybir.dt.float32
FP32R = mybir.dt.float32r
BF16 = mybir.dt.bfloat16
AX = mybir.AxisListType
ALU = mybir.AluOpType
ACTF = mybir.ActivationFunctionType

EPS = 1e-6
NEG_BIG = -1e30


def _bcast_ap(ap, nparts=128):
    """Partition-broadcast view of a DRAM AP (step-0 partition dim)."""
    return bass.AP(tensor=ap.tensor, offset=ap.offset, ap=[[0, nparts], *ap.ap])


def build_moe_nc(D, E, T, PW=256, trn_type="TRN2", expert_bf16=False):
    """Emit the per-core MoE program. Returns a compiled Bacc instance.

    D: model dim (multiple of 128); E: num experts; T: tokens per core;
    PW: weight panel width (multiple of 128, >=256 for f32r full rate on proj).
    """
    P = 128
    KO = D // P          # contraction k-tiles
    NTT = T // P         # token tiles of 128
    TOKMM = min(512, T)  # moving-operand token chunk for expert matmuls
    NTH = T // TOKMM     # token chunks
    NCP = D // PW        # weight panels (expert cols / proj cols)
    NCT = PW // P        # col-tiles of 128 per panel

    nc = bacc.Bacc(trn_type, target_bir_lowering=False, debug=False)

    xt = nc.dram_tensor("xt", [D, T], FP32, kind="ExternalInput").ap()
    xtb = (nc.dram_tensor("xtb", [D, T], BF16, kind="ExternalInput").ap()
           if expert_bf16 else None)
    xr = nc.dram_tensor("xr", [T, D], FP32, kind="ExternalInput").ap()
    router_w = nc.dram_tensor("router_w", [D, E], FP32, kind="ExternalInput").ap()
    router_b = nc.dram_tensor("router_b", [E], FP32, kind="ExternalInput").ap()
    ew_dt = BF16 if expert_bf16 else FP32
    expert_w = nc.dram_tensor("expert_w", [E, D, D], ew_dt, kind="ExternalInput").ap()
    expert_b = nc.dram_tensor("expert_b", [E, D], FP32, kind="ExternalInput").ap()
    proj_w = nc.dram_tensor("proj_w", [D, D], FP32, kind="ExternalInput").ap()
    proj_b = nc.dram_tensor("proj_b", [D], FP32, kind="ExternalInput").ap()
    norm_w = nc.dram_tensor("norm_w", [D], FP32, kind="ExternalInput").ap()
    out = nc.dram_tensor("out", [T, D], FP32, kind="ExternalOutput").ap()
    fw_dram = nc.dram_tensor("fw_scratch", [E, T], FP32).ap()

    xt_r = xt.rearrange("(ko p) t -> p ko t", p=P)
    rw_r = router_w.rearrange("(ko p) e -> p ko e", p=P)

    with tile.TileContext(nc) as tc, ExitStack() as ctx:
        v = nc.vector
        s = nc.scalar

        big = ctx.enter_context(tc.tile_pool(name="big", bufs=1))
        ct_pool = ctx.enter_context(tc.tile_pool(name="ct_pool", bufs=1))
        w_pool = ctx.enter_context(tc.tile_pool(name="w_pool", bufs=2))
        sil_pool = ctx.enter_context(tc.tile_pool(name="sil_pool", bufs=3))
        small = ctx.enter_context(tc.tile_pool(name="small", bufs=2))
        singles = ctx.enter_context(tc.tile_pool(name="singles", bufs=1))
        xres_pool = ctx.enter_context(tc.tile_pool(name="xres_pool", bufs=2))

        # ---- resident loads ----
        if expert_bf16:
            xmm_sb = big.tile([P, KO, T], BF16, tag="big", name="xtb_sb")
            xtb_r = xtb.rearrange("(ko p) t -> p ko t", p=P)
            for ko in range(KO):
                nc.sync.dma_start(out=xmm_sb[:, ko, :], in_=xtb_r[:, ko, :])
            rxt_pool = ctx.enter_context(tc.tile_pool(name="rxt_pool", bufs=1))
        else:
            xmm_sb = big.tile([P, KO, T], FP32R, tag="big", name="xt_sb")
            for ko in range(KO):
                nc.sync.dma_start(
                    out=xmm_sb[:, ko, :], in_=xt_r[:, ko, :].bitcast(FP32R)
                )

        rw_sb = singles.tile([P, KO, E], FP32)
        nc.sync.dma_start(out=rw_sb, in_=rw_r)
        rb_rep = singles.tile([P, E], FP32)
        nc.sync.dma_start(out=rb_rep, in_=_bcast_ap(router_b))
        nw_rep = singles.tile([P, D], FP32)
        nc.sync.dma_start(out=nw_rep, in_=_bcast_ap(norm_w))
        prb_rep = singles.tile([P, D], FP32)
        nc.sync.dma_start(out=prb_rep, in_=_bcast_ap(proj_b))

        identity = singles.tile([P, P], FP32)
        make_identity(nc, identity)
        eps_t = singles.tile([P, 1], FP32)
        v.memset(eps_t, EPS)

        fwT = singles.tile([E, T], FP32)  # gate weights, expert-major
        ct = ct_pool.tile([P, KO, T], FP32R)  # C^T accumulator [D, T]

        # ---- router + top-2 softmax gates ----
        with (
            tc.tile_pool(name="psr", bufs=2, space="PSUM") as psr,
            tc.tile_pool(name="pst", bufs=2, space="PSUM") as pst,
            tc.tile_pool(name="rsm", bufs=2) as rsm,
        ):
            for tt in range(NTT):
                if expert_bf16:
                    xtf = rxt_pool.tile([P, KO, P], FP32, tag="rxt")
                    nc.sync.dma_start(
                        out=xtf, in_=xt_r[:, :, tt * P:(tt + 1) * P]
                    )
                else:
                    xtf = xmm_sb[:, :, tt * P:(tt + 1) * P].bitcast(FP32)
                ps_l = psr.tile([P, E], FP32)
                for ko in range(KO):
                    nc.tensor.matmul(
                        ps_l,
                        lhsT=xtf[:, ko, :],
                        rhs=rw_sb[:, ko, :],
                        start=(ko == 0),
                        stop=(ko == KO - 1),
                    )
                logits = rsm.tile([P, E], FP32)
                v.tensor_tensor(out=logits, in0=ps_l, in1=rb_rep, op=ALU.add)
                m1 = rsm.tile([P, 1], FP32)
                v.tensor_reduce(m1, logits, axis=AX.X, op=ALU.max)
                mask1 = rsm.tile([P, E], FP32)
                v.tensor_scalar(mask1, logits, m1, None, op0=ALU.is_ge)
                lg2 = rsm.tile([P, E], FP32)
                v.scalar_tensor_tensor(
                    out=lg2, in0=mask1, scalar=NEG_BIG, in1=logits,
                    op0=ALU.mult, op1=ALU.add,
                )
                m2 = rsm.tile([P, 1], FP32)
                v.tensor_reduce(m2, lg2, axis=AX.X, op=ALU.max)
                mask2 = rsm.tile([P, E], FP32)
                v.tensor_scalar(mask2, lg2, m2, None, op0=ALU.is_ge)
                d21 = rsm.tile([P, 1], FP32)
                v.tensor_tensor(out=d21, in0=m2, in1=m1, op=ALU.subtract)
                e2 = rsm.tile([P, 1], FP32)
                s.activation(e2, d21, ACTF.Exp)
                den = rsm.tile([P, 1], FP32)
                v.tensor_scalar(den, e2, 1.0, None, op0=ALU.add)
                winv = rsm.tile([P, 1], FP32)
                v.reciprocal(winv, den)
                w2 = rsm.tile([P, 1], FP32)
                v.tensor_tensor(out=w2, in0=e2, in1=winv, op=ALU.mult)
                t2 = rsm.tile([P, E], FP32)
                v.tensor_scalar(t2, mask2, w2, None, op0=ALU.mult)
                fw = rsm.tile([P, E], FP32)
                v.scalar_tensor_tensor(
                    out=fw, in0=mask1, scalar=winv, in1=t2,
                    op0=ALU.mult, op1=ALU.add,
                )
                ps_t = pst.tile([E, P], FP32)
                nc.tensor.transpose(ps_t, fw, identity)
                v.tensor_copy(out=fwT[:, tt * P:(tt + 1) * P], in_=ps_t)
            nc.sync.dma_start(out=fw_dram, in_=fwT)

        # ---- expert phase: ct[d, t] = sum_e gate[e,t] * silu(x @ We + be)^T ----
        with tc.tile_pool(name="pse", bufs=8, space="PSUM") as pse:
            for e in range(E):
                fw_rep = sil_pool.tile([P, T], FP32, tag="fwrep", bufs=2)
                nc.sync.dma_start(out=fw_rep, in_=_bcast_ap(fw_dram[e]))
                eb_sb = small.tile([P, KO], FP32)
                nc.sync.dma_start(
                    out=eb_sb, in_=expert_b[e].rearrange("(ko p) -> p ko", p=P)
                )
                we_r = expert_w[e].rearrange("(ko p) c -> p ko c", p=P)
                for cq in range(NCP):
                    if expert_bf16:
                        wp = w_pool.tile([P, KO, PW], BF16, tag="wpb")
                        nc.sync.dma_start(
                            out=wp, in_=we_r[:, :, cq * PW:(cq + 1) * PW]
                        )
                    else:
                        wp = w_pool.tile([P, KO, PW], FP32R, tag="wp")
                        nc.sync.dma_start(
                            out=wp,
                            in_=we_r[:, :, cq * PW:(cq + 1) * PW].bitcast(FP32R),
                        )
                    for c2 in range(NCT):
                        colt = cq * NCT + c2
                        for th in range(NTH):
                            ps = pse.tile([P, TOKMM], FP32)
                            for ko in range(KO):
                                nc.tensor.matmul(
                                    ps,
                                    lhsT=wp[:, ko, c2 * P:(c2 + 1) * P],
                                    rhs=xmm_sb[:, ko, th * TOKMM:(th + 1) * TOKMM],
                                    start=(ko == 0),
                                    stop=(ko == KO - 1),
                                )
                            sg = sil_pool.tile([P, TOKMM], FP32, tag="sg")
                            s.activation(
                                sg, ps, ACTF.Sigmoid, bias=eb_sb[:, colt:colt + 1]
                            )
                            sil = sil_pool.tile([P, TOKMM], FP32, tag="sil")
                            v.scalar_tensor_tensor(
                                out=sil, in0=ps, scalar=eb_sb[:, colt:colt + 1],
                                in1=sg, op0=ALU.add, op1=ALU.mult,
                            )
                            ct_sl = ct[:, colt, th * TOKMM:(th + 1) * TOKMM]
                            fw_sl = fw_rep[:, th * TOKMM:(th + 1) * TOKMM]
                            if e == 0:
                                v.tensor_tensor(
                                    out=ct_sl, in0=sil, in1=fw_sl, op=ALU.mult
                                )
                            else:
                                v.tensor_tensor(
                                    out=sil, in0=sil, in1=fw_sl, op=ALU.mult
                                )
                                v.tensor_tensor(
                                    out=ct_sl, in0=ct_sl, in1=sil, op=ALU.add
                                )

        # ---- projection + residual into Y (token-major), reusing xt's slot ----
        y_all = big.tile([P, NTT, D], FP32, tag="big")
        pw_r = proj_w.rearrange("(ko p) c -> p ko c", p=P)
        with (
            tc.tile_pool(name="psp", bufs=4, space="PSUM") as psp,
            tc.tile_pool(name="nsm", bufs=2) as nsm,
        ):
            HD = D // 2

            def emit_norm(tt):
                # RMS norm (in place on Y[tt]) + store, interleaved with proj
                y_t = y_all[:, tt, :]
                sq = nsm.tile([P, HD], FP32, tag="sq", bufs=1, name=f"sq{tt}")
                ssa = nsm.tile([P, 1], FP32, tag="ssa", name=f"ssa{tt}")
                ssb = nsm.tile([P, 1], FP32, tag="ssb", name=f"ssb{tt}")
                v.scalar_tensor_tensor(
                    out=sq, in0=y_t[:, :HD], scalar=1.0, in1=y_t[:, :HD],
                    op0=ALU.bypass, op1=ALU.mult, accum_out=ssa,
                )
                v.scalar_tensor_tensor(
                    out=sq, in0=y_t[:, HD:], scalar=1.0, in1=y_t[:, HD:],
                    op0=ALU.bypass, op1=ALU.mult, accum_out=ssb,
                )
                ssum = nsm.tile([P, 1], FP32, tag="ssum", name=f"ssum{tt}")
                v.tensor_tensor(out=ssum, in0=ssa, in1=ssb, op=ALU.add)
                rms = nsm.tile([P, 1], FP32, tag="rms", name=f"rms{tt}")
                s.activation(rms, ssum, ACTF.Sqrt, bias=eps_t, scale=1.0 / D)
                rinv = nsm.tile([P, 1], FP32, tag="rinv", name=f"rinv{tt}")
                v.reciprocal(rinv, rms)
                v.scalar_tensor_tensor(
                    out=y_t, in0=y_t, scalar=rinv, in1=nw_rep,
                    op0=ALU.mult, op1=ALU.mult,
                )
                nc.sync.dma_start(out=out[tt * P:(tt + 1) * P, :], in_=y_t)

            for pp in range(NCP):
                pwp = w_pool.tile([P, KO, PW], FP32R, tag="wp")
                nc.sync.dma_start(out=pwp, in_=pw_r[:, :, pp * PW:(pp + 1) * PW].bitcast(FP32R))
                for tt in range(NTT):
                    ps_o = psp.tile([P, PW], FP32)
                    for ko in range(KO):
                        nc.tensor.matmul(
                            ps_o,
                            lhsT=ct[:, ko, tt * P:(tt + 1) * P],
                            rhs=pwp[:, ko, :],
                            start=(ko == 0),
                            stop=(ko == KO - 1),
                        )
                    xres = xres_pool.tile([P, PW], FP32)
                    nc.sync.dma_start(
                        out=xres,
                        in_=xr[tt * P:(tt + 1) * P, pp * PW:(pp + 1) * PW],
                    )
                    y_sl = y_all[:, tt, pp * PW:(pp + 1) * PW]
                    v.tensor_tensor(
                        out=y_sl, in0=ps_o, in1=prb_rep[:, pp * PW:(pp + 1) * PW],
                        op=ALU.add,
                    )
                    v.tensor_tensor(out=y_sl, in0=y_sl, in1=xres, op=ALU.add)
                    if pp == NCP - 1:
                        emit_norm(tt)

    nc.compile()
    return nc


# ---- full-problem entry point ----
_B, _S, _D, _E = 4, 2048, 2048, 8
_NCORES = 8
_T = _B * _S // _NCORES

_EXPERT_BF16 = False

_nc_cache = None


def _get_nc():
    global _nc_cache
    if _nc_cache is None:
        _nc_cache = build_moe_nc(_D, _E, _T, expert_bf16=_EXPERT_BF16)
    return _nc_cache


def _make_in_maps(xf, router_w, router_b, expert_w, expert_b, proj_w, proj_b,
                  norm_w):
    if _EXPERT_BF16:
        import ml_dtypes
        expert_w_c = expert_w.astype(ml_dtypes.bfloat16)
    else:
        expert_w_c = expert_w
    in_maps = []
    for c in range(_NCORES):
        xs = xf[c * _T:(c + 1) * _T]
        xst = np.ascontiguousarray(xs.T)
        m = {
            "xt": xst,
            "xr": np.ascontiguousarray(xs),
            "router_w": router_w,
            "router_b": router_b,
            "expert_w": expert_w_c,
            "expert_b": expert_b,
            "proj_w": proj_w,
            "proj_b": proj_b,
            "norm_w": norm_w,
        }
        if _EXPERT_BF16:
            import ml_dtypes
            m["xtb"] = xst.astype(ml_dtypes.bfloat16)
        in_maps.append(m)
    return in_maps


def kernel(x, router_w, router_b, expert_w, expert_b, proj_w, proj_b, norm_w):
    from concourse import bass_utils

    x = np.asarray(x, np.float32)
    router_w = np.asarray(router_w, np.float32)
    router_b = np.asarray(router_b, np.float32)
    expert_w = np.asarray(expert_w, np.float32)
    expert_b = np.asarray(expert_b, np.float32)
    proj_w = np.asarray(proj_w, np.float32)
    proj_b = np.asarray(proj_b, np.float32)
    norm_w = np.asarray(norm_w, np.float32)

    nc = _get_nc()
    xf = x.reshape(-1, _D)
    in_maps = _make_in_maps(xf, router_w, router_b, expert_w, expert_b,
                            proj_w, proj_b, norm_w)
    res = bass_utils.run_bass_kernel_spmd(nc, in_maps, core_ids=list(range(_NCORES)))
    outs = [res.results[c]["out"] for c in range(_NCORES)]
    return np.concatenate(outs, axis=0).reshape(_B, _S, _D).astype(np.float32)
